# revision 10
# baseline (speedup 1.0000x reference)
"""Chunked-prefill paged attention kernel for Trainium2 (Bass/Tile), 8 cores.

Sharding: tensor-parallel over heads. Core i handles q heads 4i..4i+3 and
kv head i. The paged-cache scatter/gather (pure data movement, index-driven)
is resolved on the host; each core runs dense attention over the gathered
[ctx | chunk] keys/values for its kv head.

Per-core layout ("transposed scores"): q and k arrive pre-transposed from the
host ([d, seq] / [d, L]) in fp16, so the PE runs two matmul passes per tile:
  scoresT[l, q] = kT_tile (stationary) x qT (moving)     -> PSUM   (fp16)
  exp on the scalar engine (PSUM -> SBUF, fp16)
  oT[d, q]     += v_tile (stationary) x expT (moving)    -> PSUM   (fp16)
The softmax denominators are NOT a third PE pass: the vector engine
accumulates exp tiles into an SBUF fp16 accumulator (2x DVE rate for 2-byte
dtypes), and one PSUM-accumulated ones-matmul triple per (head, q-chunk)
reduces that accumulator over partitions. The unnormalized oT (fp16) and the
denominators are DMA'd out; the host does the final divide and the
[d, q] -> [q, d] transpose (cheap numpy).

Engine balance per core: ACT exp is the throughput wall (1 elem/lane/cycle
at 1.2 GHz over every score element ~ 97 us); the PE's two passes run at the
same element rate but 2.4 GHz (~101 us incl. masking). Tasks are TRIPLES of
128-l-tiles so each activation covers 1536 free elements (PSUM budget: 2x3
score banks + 1 PV-acc bank + 1 sums bank = 8). Group epilogues: the oT
copy runs immediately (frees the single PV-acc bank before the next group's
first PV needs it ~2 us later); the sums matmuls are deferred three tasks so
they never block queued QK^T work while the DVE accumulator chain drains.
"""

import numpy as np

import concourse.bacc as bacc
import concourse.bass as bass
import concourse.mybir as mybir
import concourse.tile as tile
from concourse.bass_utils import run_bass_kernel_spmd

NH, NKVH, HD = 32, 8, 128
SCALE = 0.08838834764831845  # 1/sqrt(128)
SEQ, CTX = 1024, 3072
L = CTX + SEQ  # 4096
NDEV = 8
HPD = NH // NDEV  # q heads per device
QCH = 512  # q columns per moving block (psum bank width in f32)
NQC = SEQ // QCH  # q chunks
NT = L // 128  # 32 l-tiles total
NT_CTX = CTX // 128  # 24 context l-tiles
TW = 3  # l-tiles per task (psum tile = TW banks)
NEG = -1.0e30

F32 = mybir.dt.float32
F16 = mybir.dt.float16

_CACHE = {}


def _build():
    nc = bacc.Bacc("TRN2", target_bir_lowering=False, debug=False)

    qdT = nc.dram_tensor("qdT", [HPD * HD, SEQ], F16, kind="ExternalInput")
    kdT = nc.dram_tensor("kdT", [HD, L], F16, kind="ExternalInput")
    vd = nc.dram_tensor("vd", [L, HD], F16, kind="ExternalInput")
    tri = nc.dram_tensor("tri", [128, 128], F32, kind="ExternalInput")
    od = nc.dram_tensor("od", [HPD * HD, SEQ], F16, kind="ExternalOutput")
    sums_out = nc.dram_tensor("sums", [HPD, SEQ], F32, kind="ExternalOutput")

    with tile.TileContext(nc) as tc:
        with (
            tc.tile_pool(name="big", bufs=1) as big,
            tc.tile_pool(name="small", bufs=1) as small,
            tc.tile_pool(name="expp", bufs=6) as expp,
            tc.tile_pool(name="accsb", bufs=2) as accsb,
            tc.tile_pool(name="osb", bufs=2) as osb,
            tc.tile_pool(name="scps", bufs=2, space="PSUM") as scps,
            tc.tile_pool(name="accps", bufs=2, space="PSUM") as accps,
        ):
            # ---- constants ----
            tri_sb = small.tile([128, 128], F32, tag="tri")
            nc.gpsimd.dma_start(out=tri_sb, in_=tri[:, :])

            # ---- loads: straight fp16 DMA, no cast pass ----
            # k/q interleave on the SP HWDGE ring; v + tri on the gpsimd
            # ring so the first QK^T inputs are not queued behind v. The
            # first k/q transfers are split small so the lead task's
            # operands land as early as possible.
            NKC = 4  # kT chunks (8 l-tiles each)
            kT_c = [big.tile([128, L // NKC], F16, name=f"kT{i}", tag=f"kT{i}") for i in range(NKC)]
            qT_h = [big.tile([128, SEQ], F16, name=f"qT{h}", tag=f"qT{h}") for h in range(HPD)]
            v_c = [big.tile([128, NT // 4, HD], F16, name=f"v{i}", tag=f"v{i}") for i in range(4)]
            vdr = vd.rearrange("(t p) d -> p t d", p=128)

            nc.sync.dma_start(out=kT_c[0][:, 0:384], in_=kdT[:, 0:384])
            nc.sync.dma_start(out=qT_h[0][:, 0:QCH], in_=qdT[0:128, 0:QCH])
            nc.sync.dma_start(
                out=kT_c[3][:, 0:QCH], in_=kdT[:, 3 * (L // NKC) : 3 * (L // NKC) + QCH]
            )
            nc.sync.dma_start(out=kT_c[0][:, 384:], in_=kdT[:, 384 : L // NKC])
            nc.sync.dma_start(out=qT_h[0][:, QCH:], in_=qdT[0:128, QCH:])
            nc.sync.dma_start(
                out=kT_c[1], in_=kdT[:, (L // NKC) : 2 * (L // NKC)]
            )
            nc.sync.dma_start(
                out=kT_c[3][:, QCH:],
                in_=kdT[:, 3 * (L // NKC) + QCH : 4 * (L // NKC)],
            )
            nc.sync.dma_start(
                out=kT_c[2], in_=kdT[:, 2 * (L // NKC) : 3 * (L // NKC)]
            )
            for h in range(1, HPD):
                nc.sync.dma_start(
                    out=qT_h[h], in_=qdT[h * 128 : (h + 1) * 128, :]
                )
            for i in (0, 3, 1, 2):
                sl = slice(i * (NT // 4), (i + 1) * (NT // 4))
                nc.gpsimd.dma_start(out=v_c[i], in_=vdr[:, sl, :])

            def kT_at(lt):
                return kT_c[lt // 8][:, (lt % 8) * 128 : (lt % 8 + 1) * 128]

            def v_at(lt):
                return v_c[lt // 8][:, lt % 8, :]

            def diag_b(lt, c):
                """diagonal block index of this l-tile within the q-chunk
                (0..3 => partially masked; negative/ctx => fully visible)."""
                if lt < NT_CTX:
                    return -1
                return lt - NT_CTX - 4 * c

            def start_true(lt, c):
                """first unmasked q column for this l-tile (everything —
                QK^T, exp, PV, accumulation — starts here; columns left of
                it are never computed or read)."""
                b = diag_b(lt, c)
                return max(b, 0) * 128

            # ---- task list: one flat software pipeline over all
            # (head, q-chunk, l-tile-triple) tasks, so the PE never drains
            # at group boundaries: QK^T of task p+1 is emitted before PV of
            # task p. Chunk (masked) tiles are woven early among context
            # tiles; within a task, tiles are sorted by mask start so the
            # st=0 run fuses into a single activation / accumulator add.
            tasks = []  # (h, c, [lt...], first, last)
            for h in range(HPD):
                for c in range(NQC):
                    n_chunk = 4 * c + 4
                    chunk = [NT_CTX + j for j in range(n_chunk)]
                    ctx = list(range(NT_CTX))
                    seq = []
                    for j in range(n_chunk):
                        seq += [ctx[3 + j], chunk[j]]
                    seq += ctx[3 + n_chunk :]
                    groups = [ctx[0:3]]
                    groups += [seq[i : i + TW] for i in range(0, len(seq), TW)]
                    groups = [
                        sorted(g, key=lambda lt: start_true(lt, c))
                        for g in groups
                    ]
                    for gi, g in enumerate(groups):
                        tasks.append((h, c, g, gi == 0, gi == len(groups) - 1))

            group_psum = {}  # (h, c) -> acc
            group_acc2 = {}  # (h, c) -> acc2 (SBUF fp16 partial sums)
            ex_tiles = [None] * len(tasks)
            pending_epi = []  # [(due_task_idx, h, c)]

            def st_runs(pr, c):
                """[(s0, s1, st)] maximal runs of sub-tiles with equal
                start column (st=0 tiles are sorted first, so they form
                one fused run)."""
                sts = [start_true(lt, c) for lt in pr]
                runs = []
                s0 = 0
                for s in range(1, len(pr) + 1):
                    if s == len(pr) or sts[s] != sts[s0]:
                        runs.append((s0, s, sts[s0]))
                        s0 = s
                return runs

            def emit_qkt(p):
                h, c, pr, _, _ = tasks[p]
                qmv = qT_h[h][:, c * QCH : (c + 1) * QCH]
                sc = scps.tile([128, TW, QCH], F32, tag="sc")
                ex = expp.tile([128, TW, QCH], F16, tag="ex")
                ex_tiles[p] = ex
                for s, lt in enumerate(pr):
                    st = start_true(lt, c)
                    nc.tensor.matmul(
                        sc[:, s, st:],
                        kT_at(lt),
                        qmv[:, st:],
                        start=True,
                        stop=True,
                    )
                    b = diag_b(lt, c)
                    if 0 <= b <= 3:
                        # causal mask inside the diagonal 128-block: add the
                        # lower-triangular NEG template before exp (emitted
                        # ahead of the previous task's accumulator add in
                        # DVE program order, so it does not delay exp)
                        nc.vector.tensor_add(
                            out=sc[:, s, b * 128 : (b + 1) * 128],
                            in0=sc[:, s, b * 128 : (b + 1) * 128],
                            in1=tri_sb,
                        )
                # exp per maximal equal-start run (st=0 tiles fuse)
                for s0, s1, st in st_runs(pr, c):
                    nc.scalar.activation(
                        out=ex[:, s0:s1, st:],
                        in_=sc[:, s0:s1, st:],
                        func=mybir.ActivationFunctionType.Exp,
                        scale=SCALE,
                    )

            def emit_pv(p):
                h, c, pr, first, last = tasks[p]
                if first:
                    group_psum[(h, c)] = accps.tile(
                        [128, QCH], F32, name="acc", tag="acc"
                    )
                    group_acc2[(h, c)] = accsb.tile(
                        [128, TW, QCH], F16, name="acc2", tag="acc2"
                    )
                acc = group_psum[(h, c)]
                acc2 = group_acc2[(h, c)]
                ex = ex_tiles[p]
                for s, lt in enumerate(pr):
                    st = start_true(lt, c)
                    is_first = first and s == 0
                    is_last = last and s == len(pr) - 1
                    nc.tensor.matmul(
                        acc[:, st:],
                        v_at(lt),
                        ex[:, s, st:],
                        start=is_first,
                        stop=is_last,
                    )
                # partial softmax denominators: acc2 += ex on the DVE
                # (2-byte dtype -> 2x rate), one add per equal-start run.
                # The first task of a group (all-context triple) initializes
                # by copy.
                for s0, s1, st in st_runs(pr, c):
                    if first and st == 0:
                        nc.vector.tensor_copy(
                            out=acc2[:, s0:s1, :], in_=ex[:, s0:s1, :]
                        )
                    else:
                        nc.vector.tensor_add(
                            out=acc2[:, s0:s1, st:],
                            in0=acc2[:, s0:s1, st:],
                            in1=ex[:, s0:s1, st:],
                        )
                if last:
                    # oT ships immediately (frees the single PV-acc psum
                    # bank well before the next group's first PV)
                    oT_sb = osb.tile([128, QCH], F16, tag="oT_sb")
                    nc.vector.tensor_copy(out=oT_sb, in_=acc)
                    nc.sync.dma_start(
                        out=od[
                            h * 128 : (h + 1) * 128, c * QCH : (c + 1) * QCH
                        ],
                        in_=oT_sb,
                    )
                    # sums are deferred: they wait on the full DVE
                    # accumulator chain
                    pending_epi.append((p + 3, h, c))

            def emit_sums(h, c):
                group_psum.pop((h, c))
                acc2 = group_acc2.pop((h, c))
                # reduce acc2 over partitions on the (otherwise idle)
                # gpsimd engine — no PSUM, no PE work — then fold the TW
                # slots on the DVE
                red = osb.tile([1, TW, QCH], F32, tag="red")
                nc.gpsimd.tensor_reduce(
                    out=red,
                    in_=acc2,
                    axis=mybir.AxisListType.C,
                    op=mybir.AluOpType.add,
                )
                sums_sb = osb.tile([1, QCH], F32, tag="sums_sb")
                nc.vector.tensor_add(
                    out=sums_sb, in0=red[:, 0, :], in1=red[:, 1, :]
                )
                nc.vector.tensor_add(
                    out=sums_sb, in0=sums_sb, in1=red[:, 2, :]
                )
                nc.sync.dma_start(
                    out=sums_out[h : h + 1, c * QCH : (c + 1) * QCH],
                    in_=sums_sb,
                )

            with nc.allow_low_precision(reason="fp16 softmax partial sums"):
                for p in range(len(tasks) + 4):
                    if p < len(tasks):
                        emit_qkt(p)
                    if 1 <= p <= len(tasks):
                        emit_pv(p - 1)
                    while pending_epi and pending_epi[0][0] <= p:
                        _, eh, ec = pending_epi.pop(0)
                        emit_sums(eh, ec)
    nc.compile()
    return nc


def _prep_host(q, k, v, k_cache, v_cache, slot_mapping, context_slots):
    """Resolve the paged-cache scatter+gather on the host.

    Equivalent to: cache.at[slot_mapping].set(new); gather cache[context_slots];
    concat with the new chunk.
    """
    kh = np.ascontiguousarray(k).reshape(SEQ, NKVH, HD)
    vh = np.ascontiguousarray(v).reshape(SEQ, NKVH, HD)
    sm = np.asarray(slot_mapping)
    cs = np.asarray(context_slots)

    k_ctx = np.asarray(k_cache)[cs].copy()
    v_ctx = np.asarray(v_cache)[cs].copy()
    # overwrite any context slot that the new chunk was scattered into
    order = np.argsort(sm, kind="stable")
    ss = sm[order]
    j = np.searchsorted(ss, cs)
    jc = np.minimum(j, len(ss) - 1)
    hit = ss[jc] == cs
    if hit.any():
        src = order[jc[hit]]
        k_ctx[hit] = kh[src]
        v_ctx[hit] = vh[src]

    k_all = np.concatenate([k_ctx, kh], axis=0)  # [L, NKVH, HD]
    v_all = np.concatenate([v_ctx, vh], axis=0)
    return k_all, v_all


# results of the last run (exec time etc), for the local test harness
last_results = None


def kernel(q, k, v, k_cache, v_cache, slot_mapping, context_slots):
    global last_results
    q = np.asarray(q, dtype=np.float32)
    k_all, v_all = _prep_host(
        q, np.asarray(k), np.asarray(v), k_cache, v_cache, slot_mapping, context_slots
    )

    if "nc" not in _CACHE:
        _CACHE["nc"] = _build()
    nc = _CACHE["nc"]

    tri = np.where(
        np.arange(128)[None, :] >= np.arange(128)[:, None], 0.0, NEG
    ).astype(np.float32)

    in_maps = []
    for d in range(NDEV):
        in_maps.append(
            {
                "qdT": np.ascontiguousarray(
                    q[:, d * HPD * HD : (d + 1) * HPD * HD].T
                ).astype(np.float16),
                "kdT": np.ascontiguousarray(k_all[:, d, :].T).astype(np.float16),
                "vd": np.ascontiguousarray(v_all[:, d, :]).astype(np.float16),
                "tri": tri,
            }
        )

    res = run_bass_kernel_spmd(nc, in_maps, core_ids=list(range(NDEV)))
    last_results = res

    out = np.empty((SEQ, NH * HD), dtype=np.float32)
    for d in range(NDEV):
        oT = res.results[d]["od"].astype(np.float32).reshape(HPD, HD, SEQ)
        sums = res.results[d]["sums"]  # [HPD, SEQ]
        o = oT / sums[:, None, :]  # [HPD, HD, SEQ]
        out[:, d * HPD * HD : (d + 1) * HPD * HD] = (
            o.transpose(2, 0, 1).reshape(SEQ, HPD * HD)
        )
    return out


# revision 12
# speedup vs baseline: 10.0224x; 10.0224x over previous
"""Chunked-prefill paged attention kernel for Trainium2 (Bass/Tile), 8 cores.

Sharding: tensor-parallel over heads. Core i handles q heads 4i..4i+3 and
kv head i. The paged-cache scatter/gather (pure data movement, index-driven)
is resolved on the host; each core runs dense attention over the gathered
[ctx | chunk] keys/values for its kv head.

Per-core layout ("transposed scores"): q and k arrive pre-transposed from the
host ([d, seq] / [d, L]) in fp16, so the PE runs two matmul passes per tile:
  scoresT[l, q] = kT_tile (stationary) x qT (moving)     -> PSUM   (fp16)
  exp on the scalar engine (PSUM -> SBUF, fp16)
  oT[d, q]     += v_tile (stationary) x expT (moving)    -> PSUM   (fp16)
The softmax denominators are NOT a third PE pass: the vector engine
accumulates exp tiles into an SBUF fp16 accumulator (2x DVE rate for 2-byte
dtypes), and one PSUM-accumulated ones-matmul triple per (head, q-chunk)
reduces that accumulator over partitions. The unnormalized oT (fp16) and the
denominators are DMA'd out; the host does the final divide and the
[d, q] -> [q, d] transpose (cheap numpy).

Engine balance per core: ACT exp is the throughput wall (1 elem/lane/cycle
at 1.2 GHz over every score element ~ 97 us); the PE's two passes run at the
same element rate but 2.4 GHz (~101 us incl. masking). Tasks are TRIPLES of
128-l-tiles so each activation covers 1536 free elements (PSUM budget: 2x3
score banks + 1 PV-acc bank + 1 sums bank = 8). Group epilogues: the oT
copy runs immediately (frees the single PV-acc bank before the next group's
first PV needs it ~2 us later); the sums matmuls are deferred three tasks so
they never block queued QK^T work while the DVE accumulator chain drains.
"""

import numpy as np

import concourse.bacc as bacc
import concourse.bass as bass
import concourse.mybir as mybir
import concourse.tile as tile
from concourse.bass_utils import run_bass_kernel_spmd

NH, NKVH, HD = 32, 8, 128
SCALE = 0.08838834764831845  # 1/sqrt(128)
SEQ, CTX = 1024, 3072
L = CTX + SEQ  # 4096
NDEV = 8
HPD = NH // NDEV  # q heads per device
QCH = 512  # q columns per moving block (psum bank width in f32)
NQC = SEQ // QCH  # q chunks
NT = L // 128  # 32 l-tiles total
NT_CTX = CTX // 128  # 24 context l-tiles
TW = 3  # l-tiles per task (psum tile = TW banks)
NEG = -1.0e30

F32 = mybir.dt.float32
F16 = mybir.dt.float16

_CACHE = {}


def _build():
    nc = bacc.Bacc("TRN2", target_bir_lowering=False, debug=False)

    qdT = nc.dram_tensor("qdT", [HPD * HD, SEQ], F16, kind="ExternalInput")
    kdT = nc.dram_tensor("kdT", [HD, L], F16, kind="ExternalInput")
    vd = nc.dram_tensor("vd", [L, HD], F16, kind="ExternalInput")
    tri = nc.dram_tensor("tri", [128, 128], F32, kind="ExternalInput")
    od = nc.dram_tensor("od", [HPD * HD, SEQ], F16, kind="ExternalOutput")
    sums_out = nc.dram_tensor("sums", [HPD, SEQ], F32, kind="ExternalOutput")

    with tile.TileContext(nc) as tc:
        with (
            tc.tile_pool(name="big", bufs=1) as big,
            tc.tile_pool(name="small", bufs=1) as small,
            tc.tile_pool(name="expp", bufs=6) as expp,
            tc.tile_pool(name="accsb", bufs=2) as accsb,
            tc.tile_pool(name="osb", bufs=2) as osb,
            tc.tile_pool(name="scps", bufs=2, space="PSUM") as scps,
            tc.tile_pool(name="accps", bufs=2, space="PSUM") as accps,
        ):
            # ---- constants ----
            tri_sb = small.tile([128, 128], F32, tag="tri")
            nc.gpsimd.dma_start(out=tri_sb, in_=tri[:, :])
            ones_f = small.tile([128, 1], F32, tag="ones_f")
            nc.vector.memset(ones_f, 1.0)
            ones_sb = small.tile([128, 1], F16, tag="ones")
            nc.vector.tensor_copy(out=ones_sb, in_=ones_f)

            # ---- loads: straight fp16 DMA, no cast pass ----
            # k/q interleave on the SP HWDGE ring; v + tri on the gpsimd
            # ring so the first QK^T inputs are not queued behind v. The
            # first k/q transfers are split small so the lead task's
            # operands land as early as possible.
            NKC = 4  # kT chunks (8 l-tiles each)
            kT_c = [big.tile([128, L // NKC], F16, name=f"kT{i}", tag=f"kT{i}") for i in range(NKC)]
            qT_h = [big.tile([128, SEQ], F16, name=f"qT{h}", tag=f"qT{h}") for h in range(HPD)]
            v_c = [big.tile([128, NT // 4, HD], F16, name=f"v{i}", tag=f"v{i}") for i in range(4)]
            vdr = vd.rearrange("(t p) d -> p t d", p=128)

            nc.sync.dma_start(out=kT_c[0][:, 0:384], in_=kdT[:, 0:384])
            nc.sync.dma_start(out=qT_h[0][:, 0:QCH], in_=qdT[0:128, 0:QCH])
            nc.sync.dma_start(
                out=kT_c[3][:, 0:QCH], in_=kdT[:, 3 * (L // NKC) : 3 * (L // NKC) + QCH]
            )
            nc.sync.dma_start(out=kT_c[0][:, 384:], in_=kdT[:, 384 : L // NKC])
            nc.sync.dma_start(out=qT_h[0][:, QCH:], in_=qdT[0:128, QCH:])
            nc.sync.dma_start(
                out=kT_c[1], in_=kdT[:, (L // NKC) : 2 * (L // NKC)]
            )
            nc.sync.dma_start(
                out=kT_c[3][:, QCH:],
                in_=kdT[:, 3 * (L // NKC) + QCH : 4 * (L // NKC)],
            )
            nc.sync.dma_start(
                out=kT_c[2], in_=kdT[:, 2 * (L // NKC) : 3 * (L // NKC)]
            )
            for h in range(1, HPD):
                nc.sync.dma_start(
                    out=qT_h[h], in_=qdT[h * 128 : (h + 1) * 128, :]
                )
            for i in (0, 3, 1, 2):
                sl = slice(i * (NT // 4), (i + 1) * (NT // 4))
                nc.gpsimd.dma_start(out=v_c[i], in_=vdr[:, sl, :])

            def kT_at(lt):
                return kT_c[lt // 8][:, (lt % 8) * 128 : (lt % 8 + 1) * 128]

            def v_at(lt):
                return v_c[lt // 8][:, lt % 8, :]

            def diag_b(lt, c):
                """diagonal block index of this l-tile within the q-chunk
                (0..3 => partially masked; negative/ctx => fully visible)."""
                if lt < NT_CTX:
                    return -1
                return lt - NT_CTX - 4 * c

            def start_true(lt, c):
                """first unmasked q column for this l-tile (everything —
                QK^T, exp, PV, accumulation — starts here; columns left of
                it are never computed or read)."""
                b = diag_b(lt, c)
                return max(b, 0) * 128

            # ---- task list: one flat software pipeline over all
            # (head, q-chunk, l-tile-triple) tasks, so the PE never drains
            # at group boundaries: QK^T of task p+1 is emitted before PV of
            # task p. Chunk (masked) tiles are woven early among context
            # tiles; within a task, tiles are sorted by mask start so the
            # st=0 run fuses into a single activation / accumulator add.
            tasks = []  # (h, c, [lt...], first, last)
            for h in range(HPD):
                for c in range(NQC):
                    n_chunk = 4 * c + 4
                    chunk = [NT_CTX + j for j in range(n_chunk)]
                    ctx = list(range(NT_CTX))
                    seq = []
                    for j in range(n_chunk):
                        seq += [ctx[3 + j], chunk[j]]
                    seq += ctx[3 + n_chunk :]
                    groups = [ctx[0:3]]
                    groups += [seq[i : i + TW] for i in range(0, len(seq), TW)]
                    groups = [
                        sorted(g, key=lambda lt: start_true(lt, c))
                        for g in groups
                    ]
                    for gi, g in enumerate(groups):
                        tasks.append((h, c, g, gi == 0, gi == len(groups) - 1))

            group_psum = {}  # (h, c) -> acc
            group_acc2 = {}  # (h, c) -> acc2 (SBUF fp16 partial sums)
            ex_tiles = [None] * len(tasks)
            pending_epi = []  # [(due_task_idx, h, c)]

            def st_runs(pr, c):
                """[(s0, s1, st)] maximal runs of sub-tiles with equal
                start column (st=0 tiles are sorted first, so they form
                one fused run)."""
                sts = [start_true(lt, c) for lt in pr]
                runs = []
                s0 = 0
                for s in range(1, len(pr) + 1):
                    if s == len(pr) or sts[s] != sts[s0]:
                        runs.append((s0, s, sts[s0]))
                        s0 = s
                return runs

            def emit_qkt(p):
                h, c, pr, _, _ = tasks[p]
                qmv = qT_h[h][:, c * QCH : (c + 1) * QCH]
                sc = scps.tile([128, TW, QCH], F32, tag="sc")
                ex = expp.tile([128, TW, QCH], F16, tag="ex")
                ex_tiles[p] = ex
                for s, lt in enumerate(pr):
                    st = start_true(lt, c)
                    nc.tensor.matmul(
                        sc[:, s, st:],
                        kT_at(lt),
                        qmv[:, st:],
                        start=True,
                        stop=True,
                    )
                    b = diag_b(lt, c)
                    if 0 <= b <= 3:
                        # causal mask inside the diagonal 128-block: add the
                        # lower-triangular NEG template before exp (emitted
                        # ahead of the previous task's accumulator add in
                        # DVE program order, so it does not delay exp)
                        nc.vector.tensor_add(
                            out=sc[:, s, b * 128 : (b + 1) * 128],
                            in0=sc[:, s, b * 128 : (b + 1) * 128],
                            in1=tri_sb,
                        )
                # exp per maximal equal-start run (st=0 tiles fuse)
                for s0, s1, st in st_runs(pr, c):
                    nc.scalar.activation(
                        out=ex[:, s0:s1, st:],
                        in_=sc[:, s0:s1, st:],
                        func=mybir.ActivationFunctionType.Exp,
                        scale=SCALE,
                    )

            def emit_pv(p):
                h, c, pr, first, last = tasks[p]
                if first:
                    group_psum[(h, c)] = accps.tile(
                        [128, QCH], F32, name="acc", tag="acc"
                    )
                    group_acc2[(h, c)] = accsb.tile(
                        [128, TW, QCH], F16, name="acc2", tag="acc2"
                    )
                acc = group_psum[(h, c)]
                acc2 = group_acc2[(h, c)]
                ex = ex_tiles[p]
                for s, lt in enumerate(pr):
                    st = start_true(lt, c)
                    is_first = first and s == 0
                    is_last = last and s == len(pr) - 1
                    nc.tensor.matmul(
                        acc[:, st:],
                        v_at(lt),
                        ex[:, s, st:],
                        start=is_first,
                        stop=is_last,
                    )
                # partial softmax denominators: acc2 += ex on the DVE
                # (2-byte dtype -> 2x rate), one add per equal-start run.
                # The first task of a group (all-context triple) initializes
                # by copy.
                for s0, s1, st in st_runs(pr, c):
                    if first and st == 0:
                        nc.vector.tensor_copy(
                            out=acc2[:, s0:s1, :], in_=ex[:, s0:s1, :]
                        )
                    else:
                        nc.vector.tensor_add(
                            out=acc2[:, s0:s1, st:],
                            in0=acc2[:, s0:s1, st:],
                            in1=ex[:, s0:s1, st:],
                        )
                if last:
                    # oT ships immediately (frees the single PV-acc psum
                    # bank well before the next group's first PV)
                    oT_sb = osb.tile([128, QCH], F16, tag="oT_sb")
                    nc.vector.tensor_copy(out=oT_sb, in_=acc)
                    nc.sync.dma_start(
                        out=od[
                            h * 128 : (h + 1) * 128, c * QCH : (c + 1) * QCH
                        ],
                        in_=oT_sb,
                    )
                    # sums are deferred: they wait on the full DVE
                    # accumulator chain
                    pending_epi.append((p + 3, h, c))

            def emit_sums(h, c):
                acc = group_psum.pop((h, c))
                acc2 = group_acc2.pop((h, c))
                # reduce acc2 over partitions: TW PSUM-accumulated
                # ones-matmuls. The [1, QCH] target reuses partition 0 of
                # the group's own PV-acc bank — the oT copy has already
                # drained it (WAR tracked via the overlapping region), so
                # no extra PSUM bank is needed.
                for s in range(TW):
                    nc.tensor.matmul(
                        acc[0:1, :],
                        ones_sb,
                        acc2[:, s, :],
                        start=(s == 0),
                        stop=(s == TW - 1),
                    )
                sums_sb = osb.tile([1, QCH], F32, tag="sums_sb")
                nc.vector.tensor_copy(out=sums_sb, in_=acc[0:1, :])
                nc.sync.dma_start(
                    out=sums_out[h : h + 1, c * QCH : (c + 1) * QCH],
                    in_=sums_sb,
                )

            with nc.allow_low_precision(reason="fp16 softmax partial sums"):
                for p in range(len(tasks) + 4):
                    if p < len(tasks):
                        emit_qkt(p)
                    if 1 <= p <= len(tasks):
                        emit_pv(p - 1)
                    while pending_epi and pending_epi[0][0] <= p:
                        _, eh, ec = pending_epi.pop(0)
                        emit_sums(eh, ec)
    nc.compile()
    return nc


def _prep_host(q, k, v, k_cache, v_cache, slot_mapping, context_slots):
    """Resolve the paged-cache scatter+gather on the host.

    Equivalent to: cache.at[slot_mapping].set(new); gather cache[context_slots];
    concat with the new chunk.
    """
    kh = np.ascontiguousarray(k).reshape(SEQ, NKVH, HD)
    vh = np.ascontiguousarray(v).reshape(SEQ, NKVH, HD)
    sm = np.asarray(slot_mapping)
    cs = np.asarray(context_slots)

    k_ctx = np.asarray(k_cache)[cs].copy()
    v_ctx = np.asarray(v_cache)[cs].copy()
    # overwrite any context slot that the new chunk was scattered into
    order = np.argsort(sm, kind="stable")
    ss = sm[order]
    j = np.searchsorted(ss, cs)
    jc = np.minimum(j, len(ss) - 1)
    hit = ss[jc] == cs
    if hit.any():
        src = order[jc[hit]]
        k_ctx[hit] = kh[src]
        v_ctx[hit] = vh[src]

    k_all = np.concatenate([k_ctx, kh], axis=0)  # [L, NKVH, HD]
    v_all = np.concatenate([v_ctx, vh], axis=0)
    return k_all, v_all


# results of the last run (exec time etc), for the local test harness
last_results = None


def kernel(q, k, v, k_cache, v_cache, slot_mapping, context_slots):
    global last_results
    q = np.asarray(q, dtype=np.float32)
    k_all, v_all = _prep_host(
        q, np.asarray(k), np.asarray(v), k_cache, v_cache, slot_mapping, context_slots
    )

    if "nc" not in _CACHE:
        _CACHE["nc"] = _build()
    nc = _CACHE["nc"]

    tri = np.where(
        np.arange(128)[None, :] >= np.arange(128)[:, None], 0.0, NEG
    ).astype(np.float32)

    in_maps = []
    for d in range(NDEV):
        in_maps.append(
            {
                "qdT": np.ascontiguousarray(
                    q[:, d * HPD * HD : (d + 1) * HPD * HD].T
                ).astype(np.float16),
                "kdT": np.ascontiguousarray(k_all[:, d, :].T).astype(np.float16),
                "vd": np.ascontiguousarray(v_all[:, d, :]).astype(np.float16),
                "tri": tri,
            }
        )

    res = run_bass_kernel_spmd(nc, in_maps, core_ids=list(range(NDEV)))
    last_results = res

    out = np.empty((SEQ, NH * HD), dtype=np.float32)
    for d in range(NDEV):
        oT = res.results[d]["od"].astype(np.float32).reshape(HPD, HD, SEQ)
        sums = res.results[d]["sums"]  # [HPD, SEQ]
        o = oT / sums[:, None, :]  # [HPD, HD, SEQ]
        out[:, d * HPD * HD : (d + 1) * HPD * HD] = (
            o.transpose(2, 0, 1).reshape(SEQ, HPD * HD)
        )
    return out


# revision 17
# speedup vs baseline: 11.7406x; 1.1714x over previous
"""Chunked-prefill paged attention kernel for Trainium2 (Bass/Tile), 8 cores.

Sharding: tensor-parallel over heads. Core i handles q heads 4i..4i+3 and
kv head i. The paged-cache scatter/gather (pure data movement, index-driven)
is resolved on the host; each core runs dense attention over the gathered
[ctx | chunk] keys/values for its kv head.

Per-core layout ("transposed scores"): q and k arrive pre-transposed from the
host ([d, seq] / [d, L]) in fp16, so the PE runs two matmul passes per tile:
  scoresT[l, q] = kT_tile (stationary) x qT (moving)     -> PSUM   (fp16)
  exp on the scalar engine (PSUM -> SBUF, fp16)
  oT[d, q]     += v_tile (stationary) x expT (moving)    -> PSUM   (fp16)
The softmax denominators are NOT a third PE pass: the vector engine
accumulates exp tiles into an SBUF fp16 accumulator (2x DVE rate for 2-byte
dtypes), and one PSUM-accumulated ones-matmul triple per (head, q-chunk)
reduces that accumulator over partitions. The unnormalized oT (fp16) and the
denominators are DMA'd out; the host does the final divide and the
[d, q] -> [q, d] transpose (cheap numpy).

Engine balance per core: ACT exp is the throughput wall (1 elem/lane/cycle
at 1.2 GHz over every score element ~ 97 us); the PE's two passes run at the
same element rate but 2.4 GHz (~101 us incl. masking). Tasks are TRIPLES of
128-l-tiles so each activation covers 1536 free elements (PSUM budget: 2x3
score banks + 1 PV-acc bank + 1 sums bank = 8). Group epilogues: the oT
copy runs immediately (frees the single PV-acc bank before the next group's
first PV needs it ~2 us later); the sums matmuls are deferred three tasks so
they never block queued QK^T work while the DVE accumulator chain drains.
"""

import numpy as np

import concourse.bacc as bacc
import concourse.bass as bass
import concourse.mybir as mybir
import concourse.tile as tile
from concourse.bass_utils import run_bass_kernel_spmd

NH, NKVH, HD = 32, 8, 128
SCALE = 0.08838834764831845  # 1/sqrt(128)
SEQ, CTX = 1024, 3072
L = CTX + SEQ  # 4096
NDEV = 8
HPD = NH // NDEV  # q heads per device
QCH = 512  # q columns per moving block (psum bank width in f32)
NQC = SEQ // QCH  # q chunks
NT = L // 128  # 32 l-tiles total
NT_CTX = CTX // 128  # 24 context l-tiles
TW = 2  # l-tiles per task (psum tile = TW banks)
NEG = -1.0e30

F32 = mybir.dt.float32
F16 = mybir.dt.float16

_CACHE = {}


def _build():
    nc = bacc.Bacc("TRN2", target_bir_lowering=False, debug=False)

    qdT = nc.dram_tensor("qdT", [HPD * HD, SEQ], F16, kind="ExternalInput")
    kdT = nc.dram_tensor("kdT", [HD, L], F16, kind="ExternalInput")
    vd = nc.dram_tensor("vd", [L, HD], F16, kind="ExternalInput")
    tri = nc.dram_tensor("tri", [128, 128], F32, kind="ExternalInput")
    od = nc.dram_tensor("od", [HPD * HD, SEQ], F16, kind="ExternalOutput")
    sums_out = nc.dram_tensor("sums", [HPD, SEQ], F32, kind="ExternalOutput")

    with tile.TileContext(nc) as tc:
        with (
            tc.tile_pool(name="big", bufs=1) as big,
            tc.tile_pool(name="small", bufs=1) as small,
            tc.tile_pool(name="expp", bufs=6) as expp,
            tc.tile_pool(name="accsb", bufs=2) as accsb,
            tc.tile_pool(name="osb", bufs=2) as osb,
            tc.tile_pool(name="scps", bufs=3, space="PSUM") as scps,
            tc.tile_pool(name="accps", bufs=2, space="PSUM") as accps,
        ):
            # ---- constants ----
            tri_sb = small.tile([128, 128], F32, tag="tri")
            nc.gpsimd.dma_start(out=tri_sb, in_=tri[:, :])
            ones_f = small.tile([128, 1], F32, tag="ones_f")
            nc.vector.memset(ones_f, 1.0)
            ones_sb = small.tile([128, 1], F16, tag="ones")
            nc.vector.tensor_copy(out=ones_sb, in_=ones_f)

            # ---- loads: straight fp16 DMA, no cast pass ----
            # k/q interleave on the SP HWDGE ring; v + tri on the gpsimd
            # ring so the first QK^T inputs are not queued behind v. The
            # first k/q transfers are split small so the lead task's
            # operands land as early as possible.
            NKC = 4  # kT chunks (8 l-tiles each)
            kT_c = [big.tile([128, L // NKC], F16, name=f"kT{i}", tag=f"kT{i}") for i in range(NKC)]
            qT_h = [big.tile([128, SEQ], F16, name=f"qT{h}", tag=f"qT{h}") for h in range(HPD)]
            v_c = [big.tile([128, NT // 4, HD], F16, name=f"v{i}", tag=f"v{i}") for i in range(4)]
            vdr = vd.rearrange("(t p) d -> p t d", p=128)

            nc.sync.dma_start(out=kT_c[0][:, 0:384], in_=kdT[:, 0:384])
            nc.sync.dma_start(out=qT_h[0][:, 0:QCH], in_=qdT[0:128, 0:QCH])
            nc.sync.dma_start(
                out=kT_c[3][:, 0:QCH], in_=kdT[:, 3 * (L // NKC) : 3 * (L // NKC) + QCH]
            )
            nc.sync.dma_start(out=kT_c[0][:, 384:], in_=kdT[:, 384 : L // NKC])
            nc.sync.dma_start(out=qT_h[0][:, QCH:], in_=qdT[0:128, QCH:])
            nc.sync.dma_start(
                out=kT_c[1], in_=kdT[:, (L // NKC) : 2 * (L // NKC)]
            )
            nc.sync.dma_start(
                out=kT_c[3][:, QCH:],
                in_=kdT[:, 3 * (L // NKC) + QCH : 4 * (L // NKC)],
            )
            nc.sync.dma_start(
                out=kT_c[2], in_=kdT[:, 2 * (L // NKC) : 3 * (L // NKC)]
            )
            for h in range(1, HPD):
                nc.sync.dma_start(
                    out=qT_h[h], in_=qdT[h * 128 : (h + 1) * 128, :]
                )
            for i in (0, 3, 1, 2):
                sl = slice(i * (NT // 4), (i + 1) * (NT // 4))
                nc.gpsimd.dma_start(out=v_c[i], in_=vdr[:, sl, :])

            def kT_at(lt):
                return kT_c[lt // 8][:, (lt % 8) * 128 : (lt % 8 + 1) * 128]

            def v_at(lt):
                return v_c[lt // 8][:, lt % 8, :]

            def diag_b(lt, c):
                """diagonal block index of this l-tile within the q-chunk
                (0..3 => partially masked; negative/ctx => fully visible)."""
                if lt < NT_CTX:
                    return -1
                return lt - NT_CTX - 4 * c

            def start_true(lt, c):
                """first unmasked q column for this l-tile (everything —
                QK^T, exp, PV, accumulation — starts here; columns left of
                it are never computed or read)."""
                b = diag_b(lt, c)
                return max(b, 0) * 128

            # ---- task list: one flat software pipeline over all
            # (head, q-chunk, l-tile-triple) tasks, so the PE never drains
            # at group boundaries: QK^T of task p+1 is emitted before PV of
            # task p. Chunk (masked) tiles are woven early among context
            # tiles; within a task, tiles are sorted by mask start so the
            # st=0 run fuses into a single activation / accumulator add.
            tasks = []  # (h, c, [lt...], first, last)
            for h in range(HPD):
                for c in range(NQC):
                    n_chunk = 4 * c + 4
                    chunk = [NT_CTX + j for j in range(n_chunk)]
                    ctx = list(range(NT_CTX))
                    seq = []
                    for j in range(n_chunk):
                        seq += [ctx[TW + j], chunk[j]]
                    seq += ctx[TW + n_chunk :]
                    groups = [ctx[0:TW]]
                    groups += [seq[i : i + TW] for i in range(0, len(seq), TW)]
                    groups = [
                        sorted(g, key=lambda lt: start_true(lt, c))
                        for g in groups
                    ]
                    for gi, g in enumerate(groups):
                        tasks.append((h, c, g, gi == 0, gi == len(groups) - 1))

            group_psum = {}  # (h, c) -> acc
            group_acc2 = {}  # (h, c) -> acc2 (SBUF fp16 partial sums)
            ex_tiles = [None] * len(tasks)
            pending_epi = []  # [(due_task_idx, h, c)]

            def st_runs(pr, c):
                """[(s0, s1, st)] maximal runs of sub-tiles with equal
                start column (st=0 tiles are sorted first, so they form
                one fused run)."""
                sts = [start_true(lt, c) for lt in pr]
                runs = []
                s0 = 0
                for s in range(1, len(pr) + 1):
                    if s == len(pr) or sts[s] != sts[s0]:
                        runs.append((s0, s, sts[s0]))
                        s0 = s
                return runs

            def emit_qkt(p):
                h, c, pr, _, _ = tasks[p]
                qmv = qT_h[h][:, c * QCH : (c + 1) * QCH]
                sc = scps.tile([128, TW, QCH], F32, tag="sc")
                ex = expp.tile([128, TW, QCH], F16, tag="ex")
                ex_tiles[p] = ex
                for s, lt in enumerate(pr):
                    st = start_true(lt, c)
                    nc.tensor.matmul(
                        sc[:, s, st:],
                        kT_at(lt),
                        qmv[:, st:],
                        start=True,
                        stop=True,
                    )
                    b = diag_b(lt, c)
                    if 0 <= b <= 3:
                        # causal mask inside the diagonal 128-block: add the
                        # lower-triangular NEG template before exp (emitted
                        # ahead of the previous task's accumulator add in
                        # DVE program order, so it does not delay exp)
                        nc.vector.tensor_add(
                            out=sc[:, s, b * 128 : (b + 1) * 128],
                            in0=sc[:, s, b * 128 : (b + 1) * 128],
                            in1=tri_sb,
                        )
                # exp per maximal equal-start run (st=0 tiles fuse)
                for s0, s1, st in st_runs(pr, c):
                    nc.scalar.activation(
                        out=ex[:, s0:s1, st:],
                        in_=sc[:, s0:s1, st:],
                        func=mybir.ActivationFunctionType.Exp,
                        scale=SCALE,
                    )

            def emit_pv(p):
                h, c, pr, first, last = tasks[p]
                if first:
                    group_psum[(h, c)] = accps.tile(
                        [128, QCH], F32, name="acc", tag="acc"
                    )
                    group_acc2[(h, c)] = accsb.tile(
                        [128, TW, QCH], F16, name="acc2", tag="acc2"
                    )
                acc = group_psum[(h, c)]
                acc2 = group_acc2[(h, c)]
                ex = ex_tiles[p]
                for s, lt in enumerate(pr):
                    st = start_true(lt, c)
                    is_first = first and s == 0
                    is_last = last and s == len(pr) - 1
                    nc.tensor.matmul(
                        acc[:, st:],
                        v_at(lt),
                        ex[:, s, st:],
                        start=is_first,
                        stop=is_last,
                    )
                # partial softmax denominators: acc2 += ex on the DVE
                # (2-byte dtype -> 2x rate), one add per equal-start run.
                # The first task of a group (all-context triple) initializes
                # by copy.
                for s0, s1, st in st_runs(pr, c):
                    if first and st == 0:
                        nc.vector.tensor_copy(
                            out=acc2[:, s0:s1, :], in_=ex[:, s0:s1, :]
                        )
                    else:
                        nc.vector.tensor_add(
                            out=acc2[:, s0:s1, st:],
                            in0=acc2[:, s0:s1, st:],
                            in1=ex[:, s0:s1, st:],
                        )
                if last:
                    # oT ships immediately (frees the single PV-acc psum
                    # bank well before the next group's first PV)
                    oT_sb = osb.tile([128, QCH], F16, tag="oT_sb")
                    nc.vector.tensor_copy(out=oT_sb, in_=acc)
                    nc.sync.dma_start(
                        out=od[
                            h * 128 : (h + 1) * 128, c * QCH : (c + 1) * QCH
                        ],
                        in_=oT_sb,
                    )
                    # sums are deferred: they wait on the full DVE
                    # accumulator chain
                    pending_epi.append((p + 4, h, c))

            def emit_sums(h, c):
                acc = group_psum.pop((h, c))
                acc2 = group_acc2.pop((h, c))
                # reduce acc2 over partitions: TW PSUM-accumulated
                # ones-matmuls. The [1, QCH] target reuses partition 0 of
                # the group's own PV-acc bank — the oT copy has already
                # drained it (WAR tracked via the overlapping region), so
                # no extra PSUM bank is needed.
                for s in range(TW):
                    nc.tensor.matmul(
                        acc[0:1, :],
                        ones_sb,
                        acc2[:, s, :],
                        start=(s == 0),
                        stop=(s == TW - 1),
                    )
                sums_sb = osb.tile([1, QCH], F32, tag="sums_sb")
                nc.vector.tensor_copy(out=sums_sb, in_=acc[0:1, :])
                nc.sync.dma_start(
                    out=sums_out[h : h + 1, c * QCH : (c + 1) * QCH],
                    in_=sums_sb,
                )

            with nc.allow_low_precision(reason="fp16 softmax partial sums"):
                for p in range(len(tasks) + 5):
                    if p < len(tasks):
                        emit_qkt(p)
                    if 1 <= p <= len(tasks):
                        emit_pv(p - 1)
                    while pending_epi and pending_epi[0][0] <= p:
                        _, eh, ec = pending_epi.pop(0)
                        emit_sums(eh, ec)
    nc.compile()
    return nc


def _prep_host(q, k, v, k_cache, v_cache, slot_mapping, context_slots):
    """Resolve the paged-cache scatter+gather on the host.

    Equivalent to: cache.at[slot_mapping].set(new); gather cache[context_slots];
    concat with the new chunk.
    """
    kh = np.ascontiguousarray(k).reshape(SEQ, NKVH, HD)
    vh = np.ascontiguousarray(v).reshape(SEQ, NKVH, HD)
    sm = np.asarray(slot_mapping)
    cs = np.asarray(context_slots)

    k_ctx = np.asarray(k_cache)[cs].copy()
    v_ctx = np.asarray(v_cache)[cs].copy()
    # overwrite any context slot that the new chunk was scattered into
    order = np.argsort(sm, kind="stable")
    ss = sm[order]
    j = np.searchsorted(ss, cs)
    jc = np.minimum(j, len(ss) - 1)
    hit = ss[jc] == cs
    if hit.any():
        src = order[jc[hit]]
        k_ctx[hit] = kh[src]
        v_ctx[hit] = vh[src]

    k_all = np.concatenate([k_ctx, kh], axis=0)  # [L, NKVH, HD]
    v_all = np.concatenate([v_ctx, vh], axis=0)
    return k_all, v_all


# results of the last run (exec time etc), for the local test harness
last_results = None


def kernel(q, k, v, k_cache, v_cache, slot_mapping, context_slots):
    global last_results
    q = np.asarray(q, dtype=np.float32)
    k_all, v_all = _prep_host(
        q, np.asarray(k), np.asarray(v), k_cache, v_cache, slot_mapping, context_slots
    )

    if "nc" not in _CACHE:
        _CACHE["nc"] = _build()
    nc = _CACHE["nc"]

    tri = np.where(
        np.arange(128)[None, :] >= np.arange(128)[:, None], 0.0, NEG
    ).astype(np.float32)

    in_maps = []
    for d in range(NDEV):
        in_maps.append(
            {
                "qdT": np.ascontiguousarray(
                    q[:, d * HPD * HD : (d + 1) * HPD * HD].T
                ).astype(np.float16),
                "kdT": np.ascontiguousarray(k_all[:, d, :].T).astype(np.float16),
                "vd": np.ascontiguousarray(v_all[:, d, :]).astype(np.float16),
                "tri": tri,
            }
        )

    res = run_bass_kernel_spmd(nc, in_maps, core_ids=list(range(NDEV)))
    last_results = res

    out = np.empty((SEQ, NH * HD), dtype=np.float32)
    for d in range(NDEV):
        oT = res.results[d]["od"].astype(np.float32).reshape(HPD, HD, SEQ)
        sums = res.results[d]["sums"]  # [HPD, SEQ]
        o = oT / sums[:, None, :]  # [HPD, HD, SEQ]
        out[:, d * HPD * HD : (d + 1) * HPD * HD] = (
            o.transpose(2, 0, 1).reshape(SEQ, HPD * HD)
        )
    return out
